# revision 8
# baseline (speedup 1.0000x reference)
"""GBST pooling kernel for Trainium2 (Bass/Tile), 8-core data-parallel.

Problem (per batch b, data-parallel over 8 cores):
    x [T=8192, D=512] f32, W [K=4, D] f32
    pooled_k[t] = mean(x[t:t+k]) (valid window, zero-padded tail)
    scores[t,k] = <pooled_k[t], W[k]>;  w = softmax_k(scores)
    out[t] = sum_k w[t,k] * pooled_k[t]

Kernel strategy (per 125-output-column time tile, fully tile-local):
    - load x_tile [128(t), 512(d)]  (3-row overlap between tiles)
    - PE-transpose x -> xT [d, t] (4x 128x128 fp32 transposes, PSUM->SBUF via ACT)
    - scores: 4 accumulating PE matmuls (lhsT = xT chunk, rhs = W chunk) -> u[t,k] = <x[t], W[k]>
    - sliding sums of u along t (via tiny partition-shifted SBUF->SBUF DMA copies),
      scale by 1/k -> scores Y[t,k]; exp/softmax smalls on ACT+DVE
    - coefficients c_j[t'] = sum_{k>j} softmax_k[t']/k  (reverse cumsum)
    - band matrix A[t, t'] = c_{t-t'}[t'] materialized via a DRAM staircase
      write/readback (A slots pre-zeroed once; the staircase cells are rewritten
      every iteration, the off-band cells stay zero forever)
    - one fp32 PE matmul out[t', d] = sum_t A[t, t'] x[t, d] does the entire
      pooling+blend; PSUM -> SBUF -> DRAM
"""

import sys

if "/opt/trn_rl_repo" not in sys.path:
    sys.path.insert(0, "/opt/trn_rl_repo")

from contextlib import ExitStack

import numpy as np

import concourse.bass as bass
import concourse.bacc as bacc_mod
import concourse.mybir as mybir
import concourse.tile as tile
from concourse.masks import make_identity

F32 = mybir.dt.float32

B, T, D, K = 8, 8192, 512, 4
N_CORES = 8
TP = 125          # output columns per tile (128 - (K-1))
NSLOT = 4         # rotating DRAM staircase slots


def build_nc(t_total=T, d_total=D, k_scales=K, split_out_copy=True):
    """Build the per-core Bass module. Same program on all 8 cores."""
    nc = bacc_mod.Bacc(None, target_bir_lowering=False)
    x_in = nc.dram_tensor("x", (t_total, d_total), F32, kind="ExternalInput")
    w_in = nc.dram_tensor("W", (k_scales, d_total), F32, kind="ExternalInput")
    out_dram = nc.dram_tensor("out", (t_total, d_total), F32, kind="ExternalOutput")

    n_tiles = (t_total + TP - 1) // TP
    n_chunks = d_total // 128

    with tile.TileContext(nc) as tc, ExitStack() as ctx:
        consts = ctx.enter_context(tc.tile_pool(name="consts", bufs=1))
        xpool = ctx.enter_context(tc.tile_pool(name="xpool", bufs=3))
        xtpool = ctx.enter_context(tc.tile_pool(name="xtpool", bufs=2))
        smalls = ctx.enter_context(tc.tile_pool(name="smalls", bufs=4))
        apool = ctx.enter_context(tc.tile_pool(name="apool", bufs=3))
        opool = ctx.enter_context(tc.tile_pool(name="opool", bufs=3))
        ppool_t = ctx.enter_context(tc.tile_pool(name="ppool_t", bufs=2, space="PSUM"))
        ppool_u = ctx.enter_context(tc.tile_pool(name="ppool_u", bufs=2, space="PSUM"))
        ppool_o = ctx.enter_context(tc.tile_pool(name="ppool_o", bufs=2, space="PSUM"))
        adram = ctx.enter_context(tc.tile_pool(name="adram", bufs=1, space="DRAM"))

        # ---- constants ----
        identity = consts.tile([128, 128], F32)
        make_identity(nc, identity)

        # W_sb[p, c, k] = W[k, 128c + p]
        w_sb = consts.tile([128, n_chunks, k_scales], F32)
        for c in range(n_chunks):
            w_src = bass.AP(
                tensor=w_in.ap().tensor,
                offset=c * 128,
                ap=[[1, 128], [d_total, k_scales]],
            )
            nc.sync.dma_start(out=w_sb[:, c, :], in_=w_src)

        # invk[:, k] = 1 / (k+1)
        invk = consts.tile([128, k_scales], F32)
        for k in range(k_scales):
            nc.gpsimd.memset(invk[:, k : k + 1], 1.0 / (k + 1))

        zero_sb = consts.tile([128, TP], F32)
        nc.gpsimd.memset(zero_sb[:], 0.0)

        # ---- DRAM staircase slots (zeroed once; off-band cells stay zero) ----
        a_slots = [adram.tile([128, TP], F32, name=f"aslot{i}", tag=f"aslot{i}") for i in range(NSLOT)]
        for sl in a_slots:
            nc.sync.dma_start(out=sl[:, :], in_=zero_sb[:])

        # ---- main loop ----
        for i in range(n_tiles):
            t0 = i * TP
            cols = min(TP, t_total - t0)          # output columns this tile
            rows = min(128, t_total - t0)         # x rows available
            last = i == n_tiles - 1

            x_tile = xpool.tile([128, d_total], F32)
            if rows < 128:
                nc.gpsimd.memset(x_tile[:], 0.0)
            nc.sync.dma_start(
                out=x_tile[0:rows, :], in_=x_in.ap()[t0 : t0 + rows, :]
            )

            # transpose x -> xT (PSUM), then ACT copy to SBUF
            xt_psum = ppool_t.tile([128, d_total], F32)
            for c in range(n_chunks):
                nc.tensor.transpose(
                    xt_psum[:, c * 128 : (c + 1) * 128],
                    x_tile[:, c * 128 : (c + 1) * 128],
                    identity,
                )
            xt_sb = xtpool.tile([128, d_total], F32)
            nc.scalar.copy(out=xt_sb[:], in_=xt_psum[:])

            # scores: u[t, k] = sum_d x[t, d] W[k, d]
            u_psum = ppool_u.tile([128, k_scales], F32)
            for c in range(n_chunks):
                nc.tensor.matmul(
                    u_psum[:, :],
                    xt_sb[:, c * 128 : (c + 1) * 128],
                    w_sb[:, c, :],
                    start=(c == 0),
                    stop=(c == n_chunks - 1),
                )
            u_sb = smalls.tile([128, k_scales], F32)
            nc.vector.tensor_copy(u_sb[:], u_psum[:])

            # partition-shifted copies of u via SBUF->SBUF DMA
            nrow = cols  # only rows [0, cols) of the smalls chain are consumed
            us = []
            for j in range(1, k_scales):
                usj = smalls.tile([128, k_scales], F32, name=f"us{j}", tag=f"us{j}")
                nc.sync.dma_start(
                    out=usj[0:nrow, j:k_scales], in_=u_sb[j : j + nrow, j:k_scales]
                )
                us.append(usj)

            # Y[t, k] = (u[t,k] + u[t+1,k] + ... + u[t+k-1,k]) / k
            y = smalls.tile([128, k_scales], F32)
            nc.vector.tensor_copy(y[0:nrow, :], u_sb[0:nrow, :])
            for j in range(1, k_scales):
                nc.vector.tensor_add(
                    y[0:nrow, j:k_scales],
                    y[0:nrow, j:k_scales],
                    us[j - 1][0:nrow, j:k_scales],
                )
            nc.vector.tensor_mul(y[0:nrow, :], y[0:nrow, :], invk[0:nrow, :])

            if last:
                # zero scores in the zero-padded tail region (t > T - k):
                # keep cell (t', q) iff t' + q <= cols - 1
                nc.gpsimd.affine_select(
                    out=y[0:nrow, :],
                    in_=y[0:nrow, :],
                    compare_op=mybir.AluOpType.is_ge,
                    fill=0.0,
                    base=cols - 1,
                    pattern=[[-1, k_scales]],
                    channel_multiplier=-1,
                )

            # softmax over k and blend coefficients
            e = smalls.tile([128, k_scales], F32)
            nc.scalar.activation(
                e[0:nrow, :], y[0:nrow, :], mybir.ActivationFunctionType.Exp
            )
            z = smalls.tile([128, 1], F32)
            nc.vector.tensor_reduce(
                z[0:nrow, :], e[0:nrow, :], axis=mybir.AxisListType.X,
                op=mybir.AluOpType.add,
            )
            r = smalls.tile([128, 1], F32)
            nc.vector.reciprocal(r[0:nrow, :], z[0:nrow, :])

            g = smalls.tile([128, k_scales], F32)
            nc.vector.tensor_mul(g[0:nrow, :], e[0:nrow, :], invk[0:nrow, :])
            if last:
                nc.gpsimd.affine_select(
                    out=g[0:nrow, :],
                    in_=g[0:nrow, :],
                    compare_op=mybir.AluOpType.is_ge,
                    fill=0.0,
                    base=cols - 1,
                    pattern=[[-1, k_scales]],
                    channel_multiplier=-1,
                )
            # reverse cumsum over k: c_j = sum_{k > j} g_k
            for j in range(k_scales - 2, -1, -1):
                nc.vector.tensor_add(
                    g[0:nrow, j : j + 1], g[0:nrow, j : j + 1], g[0:nrow, j + 1 : j + 2]
                )
            c_sb = smalls.tile([128, k_scales], F32)
            nc.vector.tensor_scalar_mul(c_sb[0:nrow, :], g[0:nrow, :], r[0:nrow, :])

            # staircase write C -> A_dram slot: cell (t'+j, t') <- c_j[t']
            # flat offset = (t'+j)*TP + t' = t'*(TP+1) + j*TP
            slot = a_slots[i % NSLOT]
            slot_ap = slot[:, :]
            stair = bass.AP(
                tensor=slot_ap.tensor,
                offset=slot_ap.offset,
                ap=[[TP + 1, cols], [TP, k_scales]],
            )
            nc.sync.dma_start(out=stair, in_=c_sb[0:cols, :])

            a_sb = apool.tile([128, TP], F32)
            nc.sync.dma_start(out=a_sb[:, :], in_=slot[:, :])

            # blend: out[t', d] = sum_t A[t, t'] x[t, d]
            o_psum = ppool_o.tile([128, d_total], F32)
            nc.tensor.matmul(
                o_psum[0:cols, :],
                a_sb[0:rows, 0:cols],
                x_tile[0:rows, :],
                start=True,
                stop=True,
            )
            o_sb = opool.tile([128, d_total], F32)
            if split_out_copy:
                h = d_total // 2
                nc.scalar.copy(out=o_sb[0:cols, 0:h], in_=o_psum[0:cols, 0:h])
                nc.vector.tensor_copy(o_sb[0:cols, h:], o_psum[0:cols, h:])
            else:
                nc.scalar.copy(out=o_sb[0:cols, :], in_=o_psum[0:cols, :])
            nc.sync.dma_start(
                out=out_dram.ap()[t0 : t0 + cols, :], in_=o_sb[0:cols, :]
            )

    nc.finalize()
    return nc


_NC_CACHE = {}


def _get_nc(t_total=T):
    if t_total not in _NC_CACHE:
        _NC_CACHE[t_total] = build_nc(t_total=t_total)
    return _NC_CACHE[t_total]


def run_spmd(x, W, trace=False, **spmd_kwargs):
    """x [B, T, D], W [K, D] -> (out [B, T, D], BassKernelResults)."""
    from concourse.bass_utils import run_bass_kernel_spmd

    x = np.ascontiguousarray(np.asarray(x, dtype=np.float32))
    W = np.ascontiguousarray(np.asarray(W, dtype=np.float32))
    assert x.shape == (B, T, D) and W.shape == (K, D), (x.shape, W.shape)
    nc = _get_nc()
    in_maps = [{"x": x[b], "W": W} for b in range(B)]
    res = run_bass_kernel_spmd(
        nc, in_maps, core_ids=list(range(N_CORES)), trace=trace, **spmd_kwargs
    )
    out = np.stack([r["out"] for r in res.results], axis=0)
    return out, res


def kernel(x, W, max_k=None, **_):
    out, _res = run_spmd(x, W)
    return out
